# revision 1
# baseline (speedup 1.0000x reference)
"""Trainium2 Bass kernel for nn_MHA_43095701848407.

MHA forward: qkv = x @ W_qkv, RoPE on q/k, causal softmax attention,
y @ W_proj.  B=4, T=2048, C=2048, 16 heads, head_dim=128, fp32.

Sharding (8 cores): tensor-parallel over heads (4 shards x 4 heads) x
data-parallel over batch (2 groups x 2 batches).  core = group*4 + shard.
Each core computes, for its 2 batches and 4 heads:
  qkv^T tiles via fp32r matmuls (x^T streamed, W resident),
  RoPE via a permutation matmul + vector combines,
  causal attention in transposed orientation (scores^T [k,q]; exp on ACT;
  column sums via ones-matmul; y^T = v_nat.T @ p^T), then the local slice
  of the output projection, producing a partial out^T [C, T] per batch.
Host sums the 4 head-shard partials per batch and transposes back.

Self-contained: shapes/sharding hardcoded; inputs full-size numpy arrays.
"""

import math
import os
import sys
import types

import numpy as np

import concourse.bass as bass
import concourse.mybir as mybir
import concourse.tile as tile
from concourse import bacc
from concourse.bass_utils import run_bass_kernel_spmd

F32 = mybir.dt.float32
F32R = mybir.dt.float32r
AF = mybir.ActivationFunctionType
ALU = mybir.AluOpType

# Problem shape (hardcoded per contract)
B, T, C = 4, 2048, 2048
H, HD = 16, 128
NCORES = 8
BGROUPS, HSHARDS = 2, 4  # batch groups x head shards
B_LOC = B // BGROUPS  # 2 batches per core
H_LOC = H // HSHARDS  # 4 heads per core
FQK = H_LOC * HD  # 512 features for q (and for k)
FV = H_LOC * HD  # 512 features for v
F_ALL = 3 * H_LOC * HD  # 1536 qkv features per core
KO = C // 128  # 16 contraction chunks
TSLAB = 512
NSLAB = T // TSLAB  # 4 t-slabs per batch
QTILE = 512
NQT = T // QTILE  # 4 q-tiles
NKB = T // 128  # 16 key blocks
SCALE = 1.0 / math.sqrt(HD)

_CACHED = {}


def _install_ntff_hook():
    """Register the axon NTFF profile hook (container's antenv lacks it)."""
    if "antenv.axon_hooks" in sys.modules:
        return
    try:
        mod = types.ModuleType("antenv.axon_hooks")
        holder = [None]
        mod.set_axon_ntff_profile_hook = lambda h: holder.__setitem__(0, h)
        mod.get_axon_ntff_profile_hook = lambda: holder[0]
        sys.modules["antenv.axon_hooks"] = mod
        import antenv

        antenv.axon_hooks = mod
        if "/root/.axon_site" not in sys.path:
            sys.path.insert(0, "/root/.axon_site")
        from trn_agent_boot.trn_boot import _ntff_profile_via_ctypes

        mod.set_axon_ntff_profile_hook(
            _ntff_profile_via_ctypes("/opt/axon/libaxon_pjrt.so")
        )
    except Exception:
        sys.modules.pop("antenv.axon_hooks", None)


def rope_perm_matrix():
    """lhsT for the rotate-half matmul: rot^T = PT.T @ q^T.
    rot[2i] = -q[2i+1], rot[2i+1] = q[2i]."""
    pt = np.zeros((HD, HD), dtype=np.float32)
    for i in range(HD // 2):
        pt[2 * i + 1, 2 * i] = -1.0
        pt[2 * i, 2 * i + 1] = 1.0
    return pt


def build_nc():
    nc = bacc.Bacc("TRN2", target_bir_lowering=False, debug=False)

    x_t = nc.dram_tensor("x_t", [B_LOC, C, T], F32R, kind="ExternalInput").ap()
    w_qkv = nc.dram_tensor("w_qkv", [C, F_ALL], F32R, kind="ExternalInput").ap()
    w_proj = nc.dram_tensor("w_proj", [FV, C], F32R, kind="ExternalInput").ap()
    sin_t = nc.dram_tensor("sin_t", [HD, T], F32, kind="ExternalInput").ap()
    cos_t = nc.dram_tensor("cos_t", [HD, T], F32, kind="ExternalInput").ap()
    pt = nc.dram_tensor("pt", [HD, HD], F32R, kind="ExternalInput").ap()
    ones_col = nc.dram_tensor("ones_col", [128, 1], F32R, kind="ExternalInput").ap()
    ones_row = nc.dram_tensor("ones_row", [1, 128], F32R, kind="ExternalInput").ap()
    out_t = nc.dram_tensor("out_t", [B_LOC, C, T], F32, kind="ExternalOutput").ap()

    with tile.TileContext(nc) as tc:
        with nc.allow_low_precision(reason="fp32r matmul inputs by design"):
            _emit(nc, tc, x_t, w_qkv, w_proj, sin_t, cos_t, pt, ones_col,
                  ones_row, out_t)
    nc.compile()
    return nc


def _emit(nc, tc, x_t, w_qkv, w_proj, sin_t, cos_t, pt, ones_col, ones_row, out_t):
    # ---- persistent scratch in DRAM ----
    with tc.tile_pool(name="dram", bufs=1, space="DRAM") as dram_pool:
        qk_dram = [
            dram_pool.tile([2 * FQK, T], F32R, name=f"qk_dram{b}") for b in range(B_LOC)
        ]
        v_dram = [
            dram_pool.tile([T, FV], F32R, name=f"v_dram{b}") for b in range(B_LOC)
        ]

        with tc.tile_pool(name="consts", bufs=1) as consts:
            pt_sb = consts.tile([HD, HD], F32R)
            nc.sync.dma_start(pt_sb, pt)
            ones_c_sb = consts.tile([128, 1], F32R)
            nc.sync.dma_start(ones_c_sb, ones_col)
            ones_r_sb = consts.tile([1, 128], F32R)
            nc.sync.dma_start(ones_r_sb, ones_row)

            _phase_qkv(nc, tc, x_t, w_qkv, sin_t, cos_t, pt_sb, qk_dram, v_dram)
            _phase_attn_proj(
                nc, tc, w_proj, qk_dram, v_dram, ones_c_sb, ones_r_sb, out_t
            )


def _phase_qkv(nc, tc, x_t, w_qkv, sin_t, cos_t, pt_sb, qk_dram, v_dram):
    """qkv^T = W.T @ x^T with RoPE on q,k; v in natural [t, f] layout."""
    with (
        tc.tile_pool(name="wpool", bufs=1) as wpool,
        tc.tile_pool(name="xpool", bufs=2) as xpool,
        tc.tile_pool(name="scpool", bufs=2) as scpool,
        tc.tile_pool(name="ropepool", bufs=2) as ropepool,
        tc.tile_pool(name="qkpsum", bufs=3, space="PSUM") as qkpsum,
        tc.tile_pool(name="rotpsum", bufs=2, space="PSUM") as rotpsum,
        tc.tile_pool(name="vpsum", bufs=2, space="PSUM") as vpsum,
    ):
        w_sb = wpool.tile([128, KO, F_ALL], F32R)
        w_src = w_qkv.rearrange("(ko p) f -> p ko f", p=128)
        for ko in range(KO):
            nc.scalar.dma_start(w_sb[:, ko, :], w_src[:, ko, :])

        for b in range(B_LOC):
            x3 = x_t[b].rearrange("(ko p) t -> p ko t", p=128)
            for js in range(NSLAB):
                first = b == 0 and js == 0
                tsl = slice(js * TSLAB, (js + 1) * TSLAB)
                x_sb = xpool.tile([128, KO, TSLAB], F32R, name="x_sb")
                if first:
                    # split by ko so the first matmuls start after ~1/16 load
                    for ko in range(KO):
                        nc.sync.dma_start(x_sb[:, ko, :], x3[:, ko, tsl])
                else:
                    nc.sync.dma_start(x_sb, x3[:, :, tsl])
                sin_sb = scpool.tile([HD, TSLAB], F32, name="sin_sb")
                nc.sync.dma_start(sin_sb, sin_t[:, tsl])
                cos_sb = scpool.tile([HD, TSLAB], F32, name="cos_sb")
                nc.sync.dma_start(cos_sb, cos_t[:, tsl])

                # q^T, k^T feature chunks (heads) with RoPE
                qk_psums = {}
                if first:
                    # ko-outer in two groups of 4 f-chunks: compute proceeds at
                    # W/x chunk-arrival pace instead of waiting for full load
                    for fg in range(2):
                        fs = [fg * 4 + i for i in range(4)]
                        pss = {
                            f: qkpsum.tile([128, TSLAB], F32, name="qk_ps")
                            for f in fs
                        }
                        for ko in range(KO):
                            for f in fs:
                                nc.tensor.matmul(
                                    pss[f],
                                    w_sb[:, ko, f * 128 : (f + 1) * 128],
                                    x_sb[:, ko, :],
                                    start=(ko == 0),
                                    stop=(ko == KO - 1),
                                )
                        qk_psums.update(pss)
                for f in range(2 * H_LOC):
                    if first:
                        ps = qk_psums[f]
                    else:
                        ps = qkpsum.tile([128, TSLAB], F32, name="qk_ps")
                        for ko in range(KO):
                            nc.tensor.matmul(
                                ps,
                                w_sb[:, ko, f * 128 : (f + 1) * 128],
                                x_sb[:, ko, :],
                                start=(ko == 0),
                                stop=(ko == KO - 1),
                            )
                    raw = ropepool.tile([128, TSLAB], F32R, name="raw")
                    nc.vector.tensor_copy(raw, ps)
                    rot_ps = rotpsum.tile([128, TSLAB], F32, name="rot_ps")
                    nc.tensor.matmul(rot_ps, pt_sb, raw, start=True, stop=True)
                    # roped = raw*cos + rot*sin
                    t1 = ropepool.tile([128, TSLAB], F32, name="t1")
                    nc.gpsimd.tensor_tensor(t1, raw, cos_sb, ALU.mult)
                    t2 = ropepool.tile([128, TSLAB], F32, name="t2")
                    nc.vector.tensor_tensor(t2, rot_ps, sin_sb, ALU.mult)
                    roped = ropepool.tile([128, TSLAB], F32R, name="roped")
                    nc.vector.tensor_tensor(roped, t1, t2, ALU.add)
                    nc.sync.dma_start(
                        qk_dram[b][f * 128 : (f + 1) * 128, tsl], roped
                    )

                # v in natural layout
                for tb in range(TSLAB // 128):
                    vps = vpsum.tile([128, FV], F32, name="v_ps")
                    for ko in range(KO):
                        nc.tensor.matmul(
                            vps,
                            x_sb[:, ko, tb * 128 : (tb + 1) * 128],
                            w_sb[:, ko, 2 * FQK : 2 * FQK + FV],
                            start=(ko == 0),
                            stop=(ko == KO - 1),
                        )
                    v_sb = ropepool.tile([128, FV], F32R, name="v_sb")
                    nc.vector.tensor_copy(v_sb, vps)
                    r0 = js * TSLAB + tb * 128
                    nc.sync.dma_start(v_dram[b][r0 : r0 + 128, :], v_sb)


def _phase_attn_proj(nc, tc, w_proj, qk_dram, v_dram, ones_c_sb, ones_r_sb, out_t):
    with (
        tc.tile_pool(name="wppool", bufs=1) as wppool,
        tc.tile_pool(name="qkvload", bufs=3) as qkvload,
        tc.tile_pool(name="ppool", bufs=6) as ppool,
        tc.tile_pool(name="ypool", bufs=B_LOC * H_LOC) as ypool,
        tc.tile_pool(name="npool", bufs=5) as npool,
        tc.tile_pool(name="opool", bufs=3) as opool,
        tc.tile_pool(name="spsum", bufs=3, space="PSUM") as spsum,
        tc.tile_pool(name="ypsum", bufs=2, space="PSUM") as ypsum,
        tc.tile_pool(name="lpsum", bufs=1, space="PSUM") as lpsum,
        tc.tile_pool(name="opsum", bufs=2, space="PSUM") as opsum,
        tc.tile_pool(name="nbounce", bufs=4, space="DRAM") as nbounce,
    ):
        wp_sb = wppool.tile([128, H_LOC, C], F32R)
        nc.sync.dma_start(wp_sb, w_proj.rearrange("(fo p) c -> p fo c", p=128))

        def emit_head_load(b, h):
            qt_sb = qkvload.tile([HD, T], F32R, name="qt_sb")
            nc.scalar.dma_start(qt_sb, qk_dram[b][h * HD : (h + 1) * HD, :])
            kt_sb = qkvload.tile([HD, T], F32R, name="kt_sb")
            nc.scalar.dma_start(
                kt_sb, qk_dram[b][FQK + h * HD : FQK + (h + 1) * HD, :]
            )
            v_sb = qkvload.tile([128, NKB, HD], F32R, name="v_sb")
            nc.scalar.dma_start(
                v_sb,
                v_dram[b].rearrange("(kb p) f -> p kb f", p=128)[
                    :, :, h * HD : (h + 1) * HD
                ],
            )
            return qt_sb, kt_sb, v_sb

        bh_pairs = [(b, h) for b in range(B_LOC) for h in range(H_LOC)]
        pending = {}
        pending[bh_pairs[0]] = emit_head_load(*bh_pairs[0])

        y_by_batch = {b: [] for b in range(B_LOC)}
        for bh_i, (b, h) in enumerate(bh_pairs):
            y_tiles = y_by_batch[b]
            if True:
                if bh_i + 1 < len(bh_pairs):
                    pending[bh_pairs[bh_i + 1]] = emit_head_load(*bh_pairs[bh_i + 1])
                qt_sb, kt_sb, v_sb = pending.pop((b, h))
                y_sb = ypool.tile([HD, T], F32R, name="y_sb")
                y_tiles.append(y_sb)

                norm_pairs = []
                for jq in range(NQT - 1, -1, -1):
                    qsl = slice(jq * QTILE, (jq + 1) * QTILE)
                    nkb = 4 * (jq + 1)
                    y_ps = ypsum.tile([HD, QTILE], F32, name="y_ps")
                    l_ps = lpsum.tile([1, QTILE], F32, name="l_ps")
                    for kb in range(nkb):
                        # diagonal blocks only touch q >= qoff within this tile
                        s_diag = kb - 4 * jq
                        qoff = 128 * s_diag if s_diag > 0 else 0
                        qn = QTILE - qoff
                        qsub = slice(jq * QTILE + qoff, (jq + 1) * QTILE)
                        s_ps = spsum.tile([128, QTILE], F32, name="s_ps")
                        nc.tensor.matmul(
                            s_ps[:, qoff:],
                            kt_sb[:, kb * 128 : (kb + 1) * 128],
                            qt_sb[:, qsub],
                            start=True,
                            stop=True,
                        )
                        p_sb = ppool.tile([128, QTILE], F32R, name="p_sb")
                        nc.scalar.activation(
                            p_sb[:, qoff:], s_ps[:, qoff:], AF.Exp, scale=SCALE
                        )
                        if s_diag >= 0:
                            # causal: keep where (q - qoff) - k >= 0 in sub-range
                            nc.gpsimd.affine_select(
                                out=p_sb[:, qoff:],
                                in_=p_sb[:, qoff:],
                                pattern=[[1, qn]],
                                compare_op=ALU.is_ge,
                                fill=0.0,
                                base=0,
                                channel_multiplier=-1,
                            )
                        nc.tensor.matmul(
                            l_ps[:, qoff:],
                            ones_c_sb,
                            p_sb[:, qoff:],
                            start=(kb == 0),
                            stop=(kb == nkb - 1),
                        )
                        nc.tensor.matmul(
                            y_ps[:, qoff:],
                            v_sb[:, kb, :],
                            p_sb[:, qoff:],
                            start=(kb == 0),
                            stop=(kb == nkb - 1),
                        )
                    # evacuate y unnormalized immediately (frees the psum bank),
                    # then normalize in place once 1/l is broadcast
                    nc.vector.tensor_copy(y_sb[:, qsl], y_ps)
                    linv = npool.tile([1, QTILE], F32, name="linv")
                    nc.vector.reciprocal_approx_fast(linv, l_ps)
                    linv_dr = nbounce.tile([1, QTILE], F32, name="linv_dr")
                    nc.sync.dma_start(linv_dr, linv)
                    bc_sb = npool.tile([128, QTILE], F32, name="bc_sb")
                    nc.sync.dma_start(bc_sb, linv_dr.to_broadcast([128, QTILE]))
                    norm_pairs.append((qsl, bc_sb))

                # normalize at head end: broadcasts already in flight, so these
                # don't block the DVE stream mid-pipeline
                for qsl_n, bc_n in norm_pairs:
                    nc.vector.tensor_tensor(
                        y_sb[:, qsl_n], y_sb[:, qsl_n], bc_n, ALU.mult
                    )

            if h != H_LOC - 1:
                continue
            # output projection for this batch: out^T = Wp.T @ y^T
            for jt in range(NQT):
                tsl = slice(jt * QTILE, (jt + 1) * QTILE)
                for co in range(C // 128):
                    o_ps = opsum.tile([128, QTILE], F32, name="o_ps")
                    for h in range(H_LOC):
                        nc.tensor.matmul(
                            o_ps,
                            wp_sb[:, h, co * 128 : (co + 1) * 128],
                            y_tiles[h][:, tsl],
                            start=(h == 0),
                            stop=(h == H_LOC - 1),
                        )
                    o_sb = opool.tile([128, QTILE], F32, name="o_sb")
                    nc.vector.tensor_copy(o_sb, o_ps)
                    nc.sync.dma_start(
                        out_t[b, co * 128 : (co + 1) * 128, tsl], o_sb
                    )


def _get_nc():
    if "nc" not in _CACHED:
        _CACHED["nc"] = build_nc()
    return _CACHED["nc"]


def kernel(x, sin, cos, W_qkv, W_proj):
    x = np.asarray(x, dtype=np.float32)
    sin = np.asarray(sin, dtype=np.float32)
    cos = np.asarray(cos, dtype=np.float32)
    W_qkv = np.asarray(W_qkv, dtype=np.float32)
    W_proj = np.asarray(W_proj, dtype=np.float32)

    sin_t = np.ascontiguousarray(sin[0, 0].T)  # [HD, T]
    cos_t = np.ascontiguousarray(cos[0, 0].T)
    pt = rope_perm_matrix()
    ones_col = np.ones((128, 1), np.float32)
    ones_row = np.ones((1, 128), np.float32)

    in_maps = []
    for g in range(BGROUPS):
        x_tg = np.ascontiguousarray(
            x[g * B_LOC : (g + 1) * B_LOC].transpose(0, 2, 1)
        )  # [B_LOC, C, T]
        for s in range(HSHARDS):
            qcols = W_qkv[:, s * FQK : (s + 1) * FQK]
            kcols = W_qkv[:, C + s * FQK : C + (s + 1) * FQK]
            vcols = W_qkv[:, 2 * C + s * FV : 2 * C + (s + 1) * FV]
            w_qkv_loc = np.ascontiguousarray(
                np.concatenate([qcols, kcols, vcols], axis=1)
            )
            w_proj_loc = np.ascontiguousarray(W_proj[s * FV : (s + 1) * FV, :])
            in_maps.append(
                {
                    "x_t": x_tg,
                    "w_qkv": w_qkv_loc,
                    "w_proj": w_proj_loc,
                    "sin_t": sin_t,
                    "cos_t": cos_t,
                    "pt": pt,
                    "ones_col": ones_col,
                    "ones_row": ones_row,
                }
            )

    trace = bool(int(os.environ.get("KERNEL_TRACE", "0")))
    if trace:
        _install_ntff_hook()
    nc = _get_nc()
    res = run_bass_kernel_spmd(
        nc, in_maps, core_ids=list(range(NCORES)), trace=trace
    )
    _CACHED["last_result"] = res

    out = np.zeros((B, T, C), dtype=np.float32)
    for g in range(BGROUPS):
        acc = np.zeros((B_LOC, C, T), dtype=np.float32)
        for s in range(HSHARDS):
            acc += res.results[g * HSHARDS + s]["out_t"]
        out[g * B_LOC : (g + 1) * B_LOC] = acc.transpose(0, 2, 1)
    return out



# revision 9
# speedup vs baseline: 1.1923x; 1.1923x over previous
"""Trainium2 Bass kernel for nn_MHA_43095701848407.

MHA forward: qkv = x @ W_qkv, RoPE on q/k, causal softmax attention,
y @ W_proj.  B=4, T=2048, C=2048, 16 heads, head_dim=128, fp32.

Sharding (8 cores): tensor-parallel over heads (4 shards x 4 heads) x
data-parallel over batch (2 groups x 2 batches).  core = group*4 + shard.

v2: fully fused single pipeline per batch, everything SBUF-resident
(no DRAM bounce of q/k/v), bf16 operands for all matmuls (fp32 psum),
software-pipelined emission so the PE never drains:
  per batch: 4 qkv t-slabs (q^T/k^T RoPE'd + v natural, kept in SBUF),
  then 4 attention q-tiles (scores^T [k,q] blocks; exp on ACT -> bf16 p;
  causal diag via gpsimd affine_select on the 128-wide triangle only;
  column sums via ones-matmul; y^T += v.T @ p), l->1/l broadcast via a
  DRAM bounce, y evacuated normalized, output projection zippered into
  the following tile's instruction stream.
Host sums the 4 head-shard partials per batch and transposes back.

Self-contained: shapes/sharding hardcoded; inputs full-size numpy arrays.
"""

import math
import os
import sys
import types

import ml_dtypes
import numpy as np

import concourse.bass as bass
import concourse.mybir as mybir
import concourse.tile as tile
from concourse import bacc
from concourse.bass_utils import run_bass_kernel_spmd

F32 = mybir.dt.float32
F32R = mybir.dt.float32r
BF16 = mybir.dt.bfloat16
AF = mybir.ActivationFunctionType
ALU = mybir.AluOpType
BF16NP = ml_dtypes.bfloat16

# Problem shape (hardcoded per contract)
B, T, C = 4, 2048, 2048
H, HD = 16, 128
NCORES = 8
BGROUPS, HSHARDS = 2, 4  # batch groups x head shards
B_LOC = B // BGROUPS  # 2 batches per core
H_LOC = H // HSHARDS  # 4 heads per core
FQK = H_LOC * HD  # 512 features for q (and for k)
FV = H_LOC * HD  # 512 features for v
F_ALL = 3 * H_LOC * HD  # 1536 qkv features per core
KO = C // 128  # 16 contraction chunks
TSLAB = 512
NSLAB = T // TSLAB  # 4 t-slabs per batch
QTILE = 512
NQT = T // QTILE  # 4 q-tiles
NKB = T // 128  # 16 key blocks
SCALE = 1.0 / math.sqrt(HD)

_CACHED = {}


def _install_ntff_hook():
    """Register the axon NTFF profile hook (container's antenv lacks it)."""
    if "antenv.axon_hooks" in sys.modules:
        return
    try:
        mod = types.ModuleType("antenv.axon_hooks")
        holder = [None]
        mod.set_axon_ntff_profile_hook = lambda h: holder.__setitem__(0, h)
        mod.get_axon_ntff_profile_hook = lambda: holder[0]
        sys.modules["antenv.axon_hooks"] = mod
        import antenv

        antenv.axon_hooks = mod
        if "/root/.axon_site" not in sys.path:
            sys.path.insert(0, "/root/.axon_site")
        from trn_agent_boot.trn_boot import _ntff_profile_via_ctypes

        mod.set_axon_ntff_profile_hook(
            _ntff_profile_via_ctypes("/opt/axon/libaxon_pjrt.so")
        )
    except Exception:
        sys.modules.pop("antenv.axon_hooks", None)


def rope_perm_matrix():
    """lhsT for the rotate-half matmul: rot^T = PT.T @ q^T.
    rot[2i] = -q[2i+1], rot[2i+1] = q[2i]."""
    pt = np.zeros((HD, HD), dtype=np.float32)
    for i in range(HD // 2):
        pt[2 * i + 1, 2 * i] = -1.0
        pt[2 * i, 2 * i + 1] = 1.0
    return pt


def build_nc():
    nc = bacc.Bacc("TRN2", target_bir_lowering=False, debug=False)

    x_t = nc.dram_tensor("x_t", [B_LOC, C, T], BF16, kind="ExternalInput").ap()
    w_qkv = nc.dram_tensor("w_qkv", [C, F_ALL], BF16, kind="ExternalInput").ap()
    w_proj = nc.dram_tensor(
        "w_proj", [HD, H_LOC, C], BF16, kind="ExternalInput"
    ).ap()
    sin_t = nc.dram_tensor("sin_t", [HD, T], F32, kind="ExternalInput").ap()
    cos_t = nc.dram_tensor("cos_t", [HD, T], F32, kind="ExternalInput").ap()
    pt = nc.dram_tensor("pt", [HD, HD], F32R, kind="ExternalInput").ap()
    ones_col = nc.dram_tensor("ones_col", [128, 1], BF16, kind="ExternalInput").ap()
    out_t = nc.dram_tensor("out_t", [B_LOC, C, T], F32, kind="ExternalOutput").ap()

    with tile.TileContext(nc) as tc:
        with nc.allow_low_precision(reason="bf16 matmul operands by design"):
            _emit(nc, tc, x_t, w_qkv, w_proj, sin_t, cos_t, pt, ones_col, out_t)
    nc.compile()
    return nc


def _emit(nc, tc, x_t, w_qkv, w_proj, sin_t, cos_t, pt, ones_col, out_t):
    with (
        tc.tile_pool(name="consts", bufs=1) as consts,
        tc.tile_pool(name="xpool", bufs=2) as xpool,
        tc.tile_pool(name="rawpool", bufs=3) as rawpool,
        tc.tile_pool(name="t1pool", bufs=2) as t1pool,
        tc.tile_pool(name="t2pool", bufs=2) as t2pool,
        tc.tile_pool(name="ppool", bufs=6) as ppool,
        tc.tile_pool(name="ypool", bufs=2) as ypool,
        tc.tile_pool(name="lpool", bufs=2) as lpool,
        tc.tile_pool(name="bcpool", bufs=2) as bcpool,
        tc.tile_pool(name="opool", bufs=3) as opool,
        tc.tile_pool(name="bigps", bufs=3, space="PSUM") as bigps,
        tc.tile_pool(name="miscps", bufs=2, space="PSUM") as miscps,
        tc.tile_pool(name="yps", bufs=2, space="PSUM") as yps,
        tc.tile_pool(name="lps", bufs=1, space="PSUM") as lps,
        tc.tile_pool(name="nbounce", bufs=4, space="DRAM") as nbounce,
    ):
        # ---- resident tiles ----
        w_sb = consts.tile([128, KO, F_ALL], BF16)
        wp_sb = consts.tile([128, H_LOC, C], BF16)
        sin_sb = consts.tile([HD, T], F32)
        cos_sb = consts.tile([HD, T], F32)
        pt_sb = consts.tile([HD, HD], F32R)
        ones_sb = consts.tile([128, 1], BF16)
        kt_sb = consts.tile([128, H_LOC, T], BF16)  # k^T per head
        qt_sb = consts.tile([128, H_LOC, T], BF16)  # q^T per head
        v_sb = consts.tile([128, NKB, FV], BF16)  # v natural [t, f]

        w_src = w_qkv.rearrange("(ko p) f -> p ko f", p=128)
        x3 = [x_t[b].rearrange("(ko p) t -> p ko t", p=128) for b in range(B_LOC)]

        # ---- initial loads: interleave so first matmuls start early ----
        nc.scalar.dma_start(pt_sb, pt)
        nc.scalar.dma_start(ones_sb, ones_col)
        nc.scalar.dma_start(sin_sb[:, 0:TSLAB], sin_t[:, 0:TSLAB])
        nc.scalar.dma_start(cos_sb[:, 0:TSLAB], cos_t[:, 0:TSLAB])
        # first x slab + W, per-ko interleaved on the sync queue
        x_first = xpool.tile([128, KO, TSLAB], BF16, name="x_sb")
        for ko in range(KO):
            nc.sync.dma_start(x_first[:, ko, :], x3[0][:, ko, 0:TSLAB])
            nc.sync.dma_start(w_sb[:, ko, :], w_src[:, ko, :])
        nc.scalar.dma_start(wp_sb, w_proj)
        nc.scalar.dma_start(sin_sb[:, TSLAB:], sin_t[:, TSLAB:])
        nc.scalar.dma_start(cos_sb[:, TSLAB:], cos_t[:, TSLAB:])

        # ---------- helpers ----------
        def rope_post(ps, f, tsl):
            """psum qk chunk -> RoPE -> bf16 into qt/kt. Returns PE rot mm
            thunk to be emitted later (software pipelined)."""
            raw = rawpool.tile([128, TSLAB], F32R, name="raw")
            nc.vector.tensor_copy(raw, ps)

            def emit_rot():
                rot_ps = miscps.tile([128, TSLAB], F32, name="misc_ps")
                nc.tensor.matmul(rot_ps, pt_sb, raw, start=True, stop=True)
                t1 = t1pool.tile([128, TSLAB], F32, name="t1")
                nc.gpsimd.tensor_tensor(t1, raw, cos_sb[:, tsl], ALU.mult)
                t2 = t2pool.tile([128, TSLAB], F32, name="t2")
                nc.vector.tensor_tensor(t2, rot_ps, sin_sb[:, tsl], ALU.mult)
                dest = (
                    qt_sb[:, f, tsl] if f < H_LOC else kt_sb[:, f - H_LOC, tsl]
                )
                nc.vector.tensor_tensor(dest, t1, t2, ALU.add)

            return emit_rot

        def emit_slab(b, js, x_sb, proj_queue, first=False):
            tsl = slice(js * TSLAB, (js + 1) * TSLAB)
            pending_rot = []
            if first:
                # ko-outer in f-groups of 3 so compute starts after the
                # first W/x ko-chunks land
                fgroups = [[0, 1, 2], [3, 4, 5], [6, 7]]
                for fg in fgroups:
                    pss = {
                        f: bigps.tile([128, TSLAB], F32, name="mm_ps") for f in fg
                    }
                    for ko in range(KO):
                        for f in fg:
                            nc.tensor.matmul(
                                pss[f],
                                w_sb[:, ko, f * 128 : (f + 1) * 128],
                                x_sb[:, ko, :],
                                start=(ko == 0),
                                stop=(ko == KO - 1),
                            )
                    for f in fg:
                        pending_rot.append(rope_post(pss[f], f, tsl))
                        if len(pending_rot) > 1:
                            pending_rot.pop(0)()
            else:
                for f in range(2 * H_LOC):
                    ps = bigps.tile([128, TSLAB], F32, name="mm_ps")
                    for ko in range(KO):
                        nc.tensor.matmul(
                            ps,
                            w_sb[:, ko, f * 128 : (f + 1) * 128],
                            x_sb[:, ko, :],
                            start=(ko == 0),
                            stop=(ko == KO - 1),
                        )
                    pending_rot.append(rope_post(ps, f, tsl))
                    if len(pending_rot) > 1:
                        pending_rot.pop(0)()
                    if proj_queue:
                        proj_queue.pop(0)()
            # v natural [t, f] chunks; remaining rot mms zippered in
            for tb in range(TSLAB // 128):
                vps = miscps.tile([128, FV], F32, name="misc_ps")
                for ko in range(KO):
                    nc.tensor.matmul(
                        vps,
                        x_sb[:, ko, tb * 128 : (tb + 1) * 128],
                        w_sb[:, ko, 2 * FQK :],
                        start=(ko == 0),
                        stop=(ko == KO - 1),
                    )
                if pending_rot:
                    pending_rot.pop(0)()
                if proj_queue:
                    proj_queue.pop(0)()
                nc.vector.tensor_copy(v_sb[:, js * 4 + tb, :], vps)
            while pending_rot:
                pending_rot.pop(0)()

        def emit_attn_tile(b, jt, proj_queue):
            """Attention for q-tile jt, all local heads; scores^T blocks.
            proj_queue: deferred projection thunks from the previous tile,
            zippered into this tile's PE stream."""
            qsl = slice(jt * QTILE, (jt + 1) * QTILE)
            nkb = 4 * (jt + 1)
            y_tile = ypool.tile([128, H_LOC, QTILE], BF16, name="y_tile")

            state = {}  # per-head psum tiles
            staged = []  # (h, kb, p, qoff) awaiting l/y emission

            def flush_one():
                h, kb, p, qoff = staged.pop(0)
                st = state[h]
                nc.tensor.matmul(
                    st["l_ps"][:, qoff:],
                    ones_sb,
                    p[:, qoff:],
                    start=(kb == 0),
                    stop=(kb == nkb - 1),
                )
                nc.tensor.matmul(
                    st["y_ps"][:, qoff:],
                    v_sb[:, kb, h * HD : (h + 1) * HD],
                    p[:, qoff:],
                    start=(kb == 0),
                    stop=(kb == nkb - 1),
                )
                if kb == nkb - 1:
                    # softmax denominator -> broadcast 1/l -> normalized evac
                    linv = lpool.tile([1, QTILE], F32, name="linv")
                    nc.vector.reciprocal_approx_fast(linv, st["l_ps"])
                    linv_dr = nbounce.tile([1, QTILE], F32, name="linv_dr")
                    nc.scalar.dma_start(linv_dr, linv)
                    bc_sb = bcpool.tile([128, QTILE], F32, name="bc_sb")
                    nc.scalar.dma_start(bc_sb, linv_dr.to_broadcast([128, QTILE]))
                    nc.vector.tensor_tensor(
                        y_tile[:, h, :], st["y_ps"], bc_sb, ALU.mult
                    )

            steps = 0
            for h in range(H_LOC):
                state[h] = {
                    "y_ps": yps.tile([HD, QTILE], F32, name="y_ps"),
                    "l_ps": lps.tile([1, QTILE], F32, name="l_ps"),
                }
                for kb in range(nkb):
                    s_diag = kb - 4 * jt
                    qoff = 128 * s_diag if s_diag > 0 else 0
                    s_ps = bigps.tile([128, QTILE], F32, name="mm_ps")
                    nc.tensor.matmul(
                        s_ps[:, qoff:],
                        kt_sb[:, h, kb * 128 : (kb + 1) * 128],
                        qt_sb[:, h, jt * QTILE + qoff : (jt + 1) * QTILE],
                        start=True,
                        stop=True,
                    )
                    p = ppool.tile([128, QTILE], BF16, name="p_sb")
                    nc.scalar.activation(
                        p[:, qoff:], s_ps[:, qoff:], AF.Exp, scale=SCALE
                    )
                    if s_diag >= 0:
                        # causal mask: only the leading 128-wide triangle of
                        # the diagonal sub-block needs masking
                        nc.gpsimd.affine_select(
                            out=p[:, qoff : qoff + 128],
                            in_=p[:, qoff : qoff + 128],
                            pattern=[[1, 128]],
                            compare_op=ALU.is_ge,
                            fill=0.0,
                            base=0,
                            channel_multiplier=-1,
                        )
                    staged.append((h, kb, p, qoff))
                    if len(staged) > 2:
                        flush_one()
                    steps += 1
                    if proj_queue and steps % 2 == 0 and steps >= 6:
                        proj_queue.pop(0)()
            while staged:
                flush_one()
            while proj_queue:
                proj_queue.pop(0)()
            return y_tile

        def make_proj_thunks(b, jt, y_tile):
            """Projection for tile jt as a list of per-co thunks."""
            tsl = slice(jt * QTILE, (jt + 1) * QTILE)

            def make(co):
                def emit():
                    o_ps = bigps.tile([128, QTILE], F32, name="mm_ps")
                    for h in range(H_LOC):
                        nc.tensor.matmul(
                            o_ps,
                            wp_sb[:, h, co * 128 : (co + 1) * 128],
                            y_tile[:, h, :],
                            start=(h == 0),
                            stop=(h == H_LOC - 1),
                        )
                    o_sb = opool.tile([128, QTILE], F32, name="o_sb")
                    nc.vector.tensor_copy(o_sb, o_ps)
                    eng = nc.sync if co % 2 == 0 else nc.scalar
                    eng.dma_start(out_t[b, co * 128 : (co + 1) * 128, tsl], o_sb)

                return emit

            return [make(co) for co in range(C // 128)]

        # ---------- main schedule ----------
        def load_x(b, js):
            x_sb = xpool.tile([128, KO, TSLAB], BF16, name="x_sb")
            nc.sync.dma_start(x_sb, x3[b][:, :, js * TSLAB : (js + 1) * TSLAB])
            return x_sb

        x_tiles = {(0, 0): x_first}
        proj_queue = []
        for b in range(B_LOC):
            for js in range(NSLAB):
                g = b * NSLAB + js
                if g + 1 < B_LOC * NSLAB:
                    nb, njs = divmod(g + 1, NSLAB)
                    x_tiles[(nb, njs)] = load_x(nb, njs)
                emit_slab(b, js, x_tiles.pop((b, js)), proj_queue, first=(g == 0))
            for jt in range(NQT):
                y_tile = emit_attn_tile(b, jt, proj_queue)
                proj_queue = make_proj_thunks(b, jt, y_tile)
        # the very last tile's projection is the kernel tail
        while proj_queue:
            proj_queue.pop(0)()


def _get_nc():
    if "nc" not in _CACHED:
        _CACHED["nc"] = build_nc()
    return _CACHED["nc"]


def _prep_in_maps(x, sin, cos, W_qkv, W_proj):
    sin_t = np.ascontiguousarray(sin[0, 0].T).astype(np.float32)  # [HD, T]
    cos_t = np.ascontiguousarray(cos[0, 0].T).astype(np.float32)
    pt = rope_perm_matrix()
    ones_col = np.ones((128, 1), BF16NP)

    in_maps = []
    for g in range(BGROUPS):
        x_tg = np.ascontiguousarray(
            x[g * B_LOC : (g + 1) * B_LOC].transpose(0, 2, 1)
        ).astype(BF16NP)  # [B_LOC, C, T]
        for s in range(HSHARDS):
            qcols = W_qkv[:, s * FQK : (s + 1) * FQK]
            kcols = W_qkv[:, C + s * FQK : C + (s + 1) * FQK]
            vcols = W_qkv[:, 2 * C + s * FV : 2 * C + (s + 1) * FV]
            w_qkv_loc = np.ascontiguousarray(
                np.concatenate([qcols, kcols, vcols], axis=1)
            ).astype(BF16NP)
            w_proj_loc = np.ascontiguousarray(
                W_proj[s * FV : (s + 1) * FV, :]
                .reshape(H_LOC, HD, C)
                .transpose(1, 0, 2)
            ).astype(BF16NP)  # [HD, H_LOC, C]
            in_maps.append(
                {
                    "x_t": x_tg,
                    "w_qkv": w_qkv_loc,
                    "w_proj": w_proj_loc,
                    "sin_t": sin_t,
                    "cos_t": cos_t,
                    "pt": pt,
                    "ones_col": ones_col,
                }
            )
    return in_maps


def kernel(x, sin, cos, W_qkv, W_proj):
    x = np.asarray(x, dtype=np.float32)
    sin = np.asarray(sin, dtype=np.float32)
    cos = np.asarray(cos, dtype=np.float32)
    W_qkv = np.asarray(W_qkv, dtype=np.float32)
    W_proj = np.asarray(W_proj, dtype=np.float32)

    in_maps = _prep_in_maps(x, sin, cos, W_qkv, W_proj)

    trace = bool(int(os.environ.get("KERNEL_TRACE", "0")))
    if trace:
        _install_ntff_hook()
    nc = _get_nc()
    res = run_bass_kernel_spmd(
        nc, in_maps, core_ids=list(range(NCORES)), trace=trace
    )
    _CACHED["last_result"] = res

    out = np.zeros((B, T, C), dtype=np.float32)
    for g in range(BGROUPS):
        acc = np.zeros((B_LOC, C, T), dtype=np.float32)
        for s in range(HSHARDS):
            acc += res.results[g * HSHARDS + s]["out_t"]
        out[g * B_LOC : (g + 1) * B_LOC] = acc.transpose(0, 2, 1)
    return out


# revision 10
# speedup vs baseline: 1.2182x; 1.0217x over previous
"""Trainium2 Bass kernel for nn_MHA_43095701848407.

MHA forward: qkv = x @ W_qkv, RoPE on q/k, causal softmax attention,
y @ W_proj.  B=4, T=2048, C=2048, 16 heads, head_dim=128, fp32.

Sharding (8 cores): tensor-parallel over heads (4 shards x 4 heads) x
data-parallel over batch (2 groups x 2 batches).  core = group*4 + shard.

v2: fully fused single pipeline per batch, everything SBUF-resident
(no DRAM bounce of q/k/v), bf16 operands for all matmuls (fp32 psum),
software-pipelined emission so the PE never drains:
  per batch: 4 qkv t-slabs (q^T/k^T RoPE'd + v natural, kept in SBUF),
  then 4 attention q-tiles (scores^T [k,q] blocks; exp on ACT -> bf16 p;
  causal diag via gpsimd affine_select on the 128-wide triangle only;
  column sums via ones-matmul; y^T += v.T @ p), l->1/l broadcast via a
  DRAM bounce, y evacuated normalized, output projection zippered into
  the following tile's instruction stream.
Host sums the 4 head-shard partials per batch and transposes back.

Self-contained: shapes/sharding hardcoded; inputs full-size numpy arrays.
"""

import math
import os
import sys
import types

import ml_dtypes
import numpy as np

import concourse.bass as bass
import concourse.mybir as mybir
import concourse.tile as tile
from concourse import bacc
from concourse.bass_utils import run_bass_kernel_spmd

F32 = mybir.dt.float32
F32R = mybir.dt.float32r
BF16 = mybir.dt.bfloat16
AF = mybir.ActivationFunctionType
ALU = mybir.AluOpType
BF16NP = ml_dtypes.bfloat16

# Problem shape (hardcoded per contract)
B, T, C = 4, 2048, 2048
H, HD = 16, 128
NCORES = 8
BGROUPS, HSHARDS = 2, 4  # batch groups x head shards
B_LOC = B // BGROUPS  # 2 batches per core
H_LOC = H // HSHARDS  # 4 heads per core
FQK = H_LOC * HD  # 512 features for q (and for k)
FV = H_LOC * HD  # 512 features for v
F_ALL = 3 * H_LOC * HD  # 1536 qkv features per core
KO = C // 128  # 16 contraction chunks
TSLAB = 512
NSLAB = T // TSLAB  # 4 t-slabs per batch
QTILE = 512
NQT = T // QTILE  # 4 q-tiles
NKB = T // 128  # 16 key blocks
SCALE = 1.0 / math.sqrt(HD)

_CACHED = {}


def _install_ntff_hook():
    """Register the axon NTFF profile hook (container's antenv lacks it)."""
    if "antenv.axon_hooks" in sys.modules:
        return
    try:
        mod = types.ModuleType("antenv.axon_hooks")
        holder = [None]
        mod.set_axon_ntff_profile_hook = lambda h: holder.__setitem__(0, h)
        mod.get_axon_ntff_profile_hook = lambda: holder[0]
        sys.modules["antenv.axon_hooks"] = mod
        import antenv

        antenv.axon_hooks = mod
        if "/root/.axon_site" not in sys.path:
            sys.path.insert(0, "/root/.axon_site")
        from trn_agent_boot.trn_boot import _ntff_profile_via_ctypes

        mod.set_axon_ntff_profile_hook(
            _ntff_profile_via_ctypes("/opt/axon/libaxon_pjrt.so")
        )
    except Exception:
        sys.modules.pop("antenv.axon_hooks", None)


def rope_perm_matrix():
    """lhsT for the rotate-half matmul: rot^T = PT.T @ q^T.
    rot[2i] = -q[2i+1], rot[2i+1] = q[2i]."""
    pt = np.zeros((HD, HD), dtype=np.float32)
    for i in range(HD // 2):
        pt[2 * i + 1, 2 * i] = -1.0
        pt[2 * i, 2 * i + 1] = 1.0
    return pt


def build_nc():
    nc = bacc.Bacc("TRN2", target_bir_lowering=False, debug=False)

    x_t = nc.dram_tensor("x_t", [B_LOC, C, T], BF16, kind="ExternalInput").ap()
    w_qkv = nc.dram_tensor("w_qkv", [C, F_ALL], BF16, kind="ExternalInput").ap()
    w_proj = nc.dram_tensor(
        "w_proj", [HD, H_LOC, C], BF16, kind="ExternalInput"
    ).ap()
    sin_t = nc.dram_tensor("sin_t", [HD, T], F32, kind="ExternalInput").ap()
    cos_t = nc.dram_tensor("cos_t", [HD, T], F32, kind="ExternalInput").ap()
    pt = nc.dram_tensor("pt", [HD, HD], F32R, kind="ExternalInput").ap()
    ones_col = nc.dram_tensor("ones_col", [128, 1], BF16, kind="ExternalInput").ap()
    out_t = nc.dram_tensor("out_t", [B_LOC, C, T], F32, kind="ExternalOutput").ap()

    with tile.TileContext(nc) as tc:
        with nc.allow_low_precision(reason="bf16 matmul operands by design"):
            _emit(nc, tc, x_t, w_qkv, w_proj, sin_t, cos_t, pt, ones_col, out_t)
    nc.compile()
    return nc


def _emit(nc, tc, x_t, w_qkv, w_proj, sin_t, cos_t, pt, ones_col, out_t):
    with (
        tc.tile_pool(name="consts", bufs=1) as consts,
        tc.tile_pool(name="xpool", bufs=2) as xpool,
        tc.tile_pool(name="rawpool", bufs=3) as rawpool,
        tc.tile_pool(name="t1pool", bufs=2) as t1pool,
        tc.tile_pool(name="t2pool", bufs=2) as t2pool,
        tc.tile_pool(name="ppool", bufs=8) as ppool,
        tc.tile_pool(name="ypool", bufs=2) as ypool,
        tc.tile_pool(name="lpool", bufs=2) as lpool,
        tc.tile_pool(name="bcpool", bufs=2) as bcpool,
        tc.tile_pool(name="opool", bufs=3) as opool,
        tc.tile_pool(name="mmps", bufs=5, space="PSUM") as mmps,
        tc.tile_pool(name="yps", bufs=2, space="PSUM") as yps,
        tc.tile_pool(name="lps", bufs=1, space="PSUM") as lps,
        tc.tile_pool(name="nbounce", bufs=4, space="DRAM") as nbounce,
    ):
        # ---- resident tiles ----
        w_sb = consts.tile([128, KO, F_ALL], BF16)
        wp_sb = consts.tile([128, H_LOC, C], BF16)
        sin_sb = consts.tile([HD, T], F32)
        cos_sb = consts.tile([HD, T], F32)
        pt_sb = consts.tile([HD, HD], F32R)
        ones_sb = consts.tile([128, 1], BF16)
        kt_sb = consts.tile([128, H_LOC, T], BF16)  # k^T per head
        qt_sb = consts.tile([128, H_LOC, T], BF16)  # q^T per head
        v_sb = consts.tile([128, NKB, FV], BF16)  # v natural [t, f]

        w_src = w_qkv.rearrange("(ko p) f -> p ko f", p=128)
        x3 = [x_t[b].rearrange("(ko p) t -> p ko t", p=128) for b in range(B_LOC)]

        # ---- initial loads ----
        # scalar queue: small consts + the first x slab (per-ko chunks)
        nc.scalar.dma_start(pt_sb, pt)
        nc.scalar.dma_start(ones_sb, ones_col)
        nc.scalar.dma_start(sin_sb[:, 0:TSLAB], sin_t[:, 0:TSLAB])
        nc.scalar.dma_start(cos_sb[:, 0:TSLAB], cos_t[:, 0:TSLAB])
        x_first = xpool.tile([128, KO, TSLAB], BF16, name="x_sb")
        for ko in range(KO):
            nc.scalar.dma_start(x_first[:, ko, :], x3[0][:, ko, 0:TSLAB])
        # sync queue: W in feature-range chunks matching the first slab's
        # f-group consumption order, so slab-0 compute paces with arrival
        for c0, c1 in ((0, 384), (384, 768), (768, 1024), (1024, F_ALL)):
            for ko in range(KO):
                nc.sync.dma_start(w_sb[:, ko, c0:c1], w_src[:, ko, c0:c1])
        nc.scalar.dma_start(sin_sb[:, TSLAB:], sin_t[:, TSLAB:])
        nc.scalar.dma_start(cos_sb[:, TSLAB:], cos_t[:, TSLAB:])
        nc.scalar.dma_start(wp_sb, w_proj)

        # ---------- helpers ----------
        def rope_post(ps, f, tsl):
            """psum qk chunk -> RoPE -> bf16 into qt/kt. Returns PE rot mm
            thunk to be emitted later (software pipelined)."""
            raw = rawpool.tile([128, TSLAB], F32R, name="raw")
            nc.vector.tensor_copy(raw, ps)

            def emit_rot():
                rot_ps = mmps.tile([128, TSLAB], F32, name="mm_ps")
                nc.tensor.matmul(rot_ps, pt_sb, raw, start=True, stop=True)
                t1 = t1pool.tile([128, TSLAB], F32, name="t1")
                nc.gpsimd.tensor_tensor(t1, raw, cos_sb[:, tsl], ALU.mult)
                t2 = t2pool.tile([128, TSLAB], F32, name="t2")
                nc.vector.tensor_tensor(t2, rot_ps, sin_sb[:, tsl], ALU.mult)
                dest = (
                    qt_sb[:, f, tsl] if f < H_LOC else kt_sb[:, f - H_LOC, tsl]
                )
                nc.vector.tensor_tensor(dest, t1, t2, ALU.add)

            return emit_rot

        def emit_slab(b, js, x_sb, proj_queue, first=False):
            tsl = slice(js * TSLAB, (js + 1) * TSLAB)
            pending_rot = []
            if first:
                # ko-outer in f-groups of 3 so compute starts after the
                # first W/x ko-chunks land
                fgroups = [[0, 1, 2], [3, 4, 5], [6, 7]]
                for fg in fgroups:
                    pss = {
                        f: mmps.tile([128, TSLAB], F32, name="mm_ps") for f in fg
                    }
                    for ko in range(KO):
                        for f in fg:
                            nc.tensor.matmul(
                                pss[f],
                                w_sb[:, ko, f * 128 : (f + 1) * 128],
                                x_sb[:, ko, :],
                                start=(ko == 0),
                                stop=(ko == KO - 1),
                            )
                    for f in fg:
                        pending_rot.append(rope_post(pss[f], f, tsl))
                        if len(pending_rot) > 1:
                            pending_rot.pop(0)()
            else:
                for f in range(2 * H_LOC):
                    ps = mmps.tile([128, TSLAB], F32, name="mm_ps")
                    for ko in range(KO):
                        nc.tensor.matmul(
                            ps,
                            w_sb[:, ko, f * 128 : (f + 1) * 128],
                            x_sb[:, ko, :],
                            start=(ko == 0),
                            stop=(ko == KO - 1),
                        )
                    pending_rot.append(rope_post(ps, f, tsl))
                    if len(pending_rot) > 1:
                        pending_rot.pop(0)()
                    if proj_queue:
                        proj_queue.pop(0)()
            # v natural [t, f] chunks; remaining rot mms zippered in
            for tb in range(TSLAB // 128):
                vps = mmps.tile([128, FV], F32, name="mm_ps")
                for ko in range(KO):
                    nc.tensor.matmul(
                        vps,
                        x_sb[:, ko, tb * 128 : (tb + 1) * 128],
                        w_sb[:, ko, 2 * FQK :],
                        start=(ko == 0),
                        stop=(ko == KO - 1),
                    )
                if pending_rot:
                    pending_rot.pop(0)()
                if proj_queue:
                    proj_queue.pop(0)()
                nc.vector.tensor_copy(v_sb[:, js * 4 + tb, :], vps)
            while pending_rot:
                pending_rot.pop(0)()

        def emit_attn_tile(b, jt, proj_queue):
            """Attention for q-tile jt, all local heads; scores^T blocks.
            proj_queue: deferred projection thunks from the previous tile,
            zippered into this tile's PE stream."""
            qsl = slice(jt * QTILE, (jt + 1) * QTILE)
            nkb = 4 * (jt + 1)
            y_tile = ypool.tile([128, H_LOC, QTILE], BF16, name="y_tile")

            state = {}  # per-head psum tiles
            staged = []  # (h, kb, p, qoff) awaiting l/y emission

            def flush_one():
                h, kb, p, qoff = staged.pop(0)
                st = state[h]
                nc.tensor.matmul(
                    st["l_ps"][:, qoff:],
                    ones_sb,
                    p[:, qoff:],
                    start=(kb == 0),
                    stop=(kb == nkb - 1),
                )
                nc.tensor.matmul(
                    st["y_ps"][:, qoff:],
                    v_sb[:, kb, h * HD : (h + 1) * HD],
                    p[:, qoff:],
                    start=(kb == 0),
                    stop=(kb == nkb - 1),
                )
                if kb == nkb - 1:
                    # softmax denominator -> broadcast 1/l -> normalized evac
                    linv = lpool.tile([1, QTILE], F32, name="linv")
                    nc.vector.reciprocal_approx_fast(linv, st["l_ps"])
                    linv_dr = nbounce.tile([1, QTILE], F32, name="linv_dr")
                    nc.scalar.dma_start(linv_dr, linv)
                    bc_sb = bcpool.tile([128, QTILE], F32, name="bc_sb")
                    nc.scalar.dma_start(bc_sb, linv_dr.to_broadcast([128, QTILE]))
                    nc.vector.tensor_tensor(
                        y_tile[:, h, :], st["y_ps"], bc_sb, ALU.mult
                    )

            steps = 0
            for h in range(H_LOC):
                state[h] = {
                    "y_ps": yps.tile([HD, QTILE], F32, name="y_ps"),
                    "l_ps": lps.tile([1, QTILE], F32, name="l_ps"),
                }
                for kb in range(nkb):
                    s_diag = kb - 4 * jt
                    qoff = 128 * s_diag if s_diag > 0 else 0
                    s_ps = mmps.tile([128, QTILE], F32, name="mm_ps")
                    nc.tensor.matmul(
                        s_ps[:, qoff:],
                        kt_sb[:, h, kb * 128 : (kb + 1) * 128],
                        qt_sb[:, h, jt * QTILE + qoff : (jt + 1) * QTILE],
                        start=True,
                        stop=True,
                    )
                    p = ppool.tile([128, QTILE], BF16, name="p_sb")
                    nc.scalar.activation(
                        p[:, qoff:], s_ps[:, qoff:], AF.Exp, scale=SCALE
                    )
                    if s_diag >= 0:
                        # causal mask: only the leading 128-wide triangle of
                        # the diagonal sub-block needs masking
                        nc.gpsimd.affine_select(
                            out=p[:, qoff : qoff + 128],
                            in_=p[:, qoff : qoff + 128],
                            pattern=[[1, 128]],
                            compare_op=ALU.is_ge,
                            fill=0.0,
                            base=0,
                            channel_multiplier=-1,
                        )
                    staged.append((h, kb, p, qoff))
                    if len(staged) > 3:
                        flush_one()
                    steps += 1
                    if proj_queue and steps % 2 == 0 and steps >= 6:
                        proj_queue.pop(0)()
            while staged:
                flush_one()
            while proj_queue:
                proj_queue.pop(0)()
            return y_tile

        def make_proj_thunks(b, jt, y_tile):
            """Projection for tile jt as a list of per-co thunks."""
            tsl = slice(jt * QTILE, (jt + 1) * QTILE)

            def make(co):
                def emit():
                    o_ps = mmps.tile([128, QTILE], F32, name="mm_ps")
                    for h in range(H_LOC):
                        nc.tensor.matmul(
                            o_ps,
                            wp_sb[:, h, co * 128 : (co + 1) * 128],
                            y_tile[:, h, :],
                            start=(h == 0),
                            stop=(h == H_LOC - 1),
                        )
                    o_sb = opool.tile([128, QTILE], F32, name="o_sb")
                    nc.vector.tensor_copy(o_sb, o_ps)
                    eng = nc.sync if co % 2 == 0 else nc.scalar
                    eng.dma_start(out_t[b, co * 128 : (co + 1) * 128, tsl], o_sb)

                return emit

            return [make(co) for co in range(C // 128)]

        # ---------- main schedule ----------
        def load_x(b, js):
            x_sb = xpool.tile([128, KO, TSLAB], BF16, name="x_sb")
            nc.sync.dma_start(x_sb, x3[b][:, :, js * TSLAB : (js + 1) * TSLAB])
            return x_sb

        x_tiles = {(0, 0): x_first}
        proj_queue = []
        for b in range(B_LOC):
            for js in range(NSLAB):
                g = b * NSLAB + js
                if g + 1 < B_LOC * NSLAB:
                    nb, njs = divmod(g + 1, NSLAB)
                    x_tiles[(nb, njs)] = load_x(nb, njs)
                emit_slab(b, js, x_tiles.pop((b, js)), proj_queue, first=(g == 0))
            for jt in range(NQT):
                y_tile = emit_attn_tile(b, jt, proj_queue)
                proj_queue = make_proj_thunks(b, jt, y_tile)
        # the very last tile's projection is the kernel tail
        while proj_queue:
            proj_queue.pop(0)()


def _get_nc():
    if "nc" not in _CACHED:
        _CACHED["nc"] = build_nc()
    return _CACHED["nc"]


def _prep_in_maps(x, sin, cos, W_qkv, W_proj):
    sin_t = np.ascontiguousarray(sin[0, 0].T).astype(np.float32)  # [HD, T]
    cos_t = np.ascontiguousarray(cos[0, 0].T).astype(np.float32)
    pt = rope_perm_matrix()
    ones_col = np.ones((128, 1), BF16NP)

    in_maps = []
    for g in range(BGROUPS):
        x_tg = np.ascontiguousarray(
            x[g * B_LOC : (g + 1) * B_LOC].transpose(0, 2, 1)
        ).astype(BF16NP)  # [B_LOC, C, T]
        for s in range(HSHARDS):
            qcols = W_qkv[:, s * FQK : (s + 1) * FQK]
            kcols = W_qkv[:, C + s * FQK : C + (s + 1) * FQK]
            vcols = W_qkv[:, 2 * C + s * FV : 2 * C + (s + 1) * FV]
            w_qkv_loc = np.ascontiguousarray(
                np.concatenate([qcols, kcols, vcols], axis=1)
            ).astype(BF16NP)
            w_proj_loc = np.ascontiguousarray(
                W_proj[s * FV : (s + 1) * FV, :]
                .reshape(H_LOC, HD, C)
                .transpose(1, 0, 2)
            ).astype(BF16NP)  # [HD, H_LOC, C]
            in_maps.append(
                {
                    "x_t": x_tg,
                    "w_qkv": w_qkv_loc,
                    "w_proj": w_proj_loc,
                    "sin_t": sin_t,
                    "cos_t": cos_t,
                    "pt": pt,
                    "ones_col": ones_col,
                }
            )
    return in_maps


def kernel(x, sin, cos, W_qkv, W_proj):
    x = np.asarray(x, dtype=np.float32)
    sin = np.asarray(sin, dtype=np.float32)
    cos = np.asarray(cos, dtype=np.float32)
    W_qkv = np.asarray(W_qkv, dtype=np.float32)
    W_proj = np.asarray(W_proj, dtype=np.float32)

    in_maps = _prep_in_maps(x, sin, cos, W_qkv, W_proj)

    trace = bool(int(os.environ.get("KERNEL_TRACE", "0")))
    if trace:
        _install_ntff_hook()
    nc = _get_nc()
    res = run_bass_kernel_spmd(
        nc, in_maps, core_ids=list(range(NCORES)), trace=trace
    )
    _CACHED["last_result"] = res

    out = np.zeros((B, T, C), dtype=np.float32)
    for g in range(BGROUPS):
        acc = np.zeros((B_LOC, C, T), dtype=np.float32)
        for s in range(HSHARDS):
            acc += res.results[g * HSHARDS + s]["out_t"]
        out[g * B_LOC : (g + 1) * B_LOC] = acc.transpose(0, 2, 1)
    return out
